# revision 38
# baseline (speedup 1.0000x reference)
"""Trainium2 Bass kernel: multi-table embedding gather (pooling=NONE).

Reference computation (hardcoded shapes):
    indices: [F=4, BL=204800] int   (values in [0, V))
    tables:  [F=4, V=1e6, D=64] f32
    out[f]   = tables[PERM[f]][indices[PERM[f]]]   -> [4, 204800, 64] f32
    PERM = [2, 0, 3, 1]

Strategy (model/table-parallel, per the sharding hint):
  * Fold the table permutation into global row ids g = PERM[f]*V + idx over a
    flat [4M, 64] table; shard row-wise across 8 cores (500,000 rows each).
  * Host routes every lookup to its owning core, bucketing by 32,768-row
    window so the gather uses the int16 `dma_gather` SWDGE ucode with
    1024-idx single-packet sub-gathers (64 descriptors/engine, the packet
    ceiling; multi-packet and prepare_only modes are ~10-100x slower per
    descriptor on the Q7).
  * RAW bass pipeline (no TileContext): Tile tracks each SWDGE DMA on one of
    8 DMASW semaphore lanes, which made every gather wait for the FULL
    completion of the gather 8 before it; the engines ping-ponged between
    ~4.5us of full-rate drain and ~5us of idle (measured 290us). Here each
    window's gathers share one explicit semaphore (+16/gather), so the Pool
    engine runs ~NBUF windows ahead and the SDMA engines stay saturated.
  * Within a window, granules guaranteed full on every core use an immediate
    count; the partial tail granule reads a preloaded count register.
  * Gathered f32 rows are cast to bf16 on the DVE and written back with one
    contiguous HWDGE DMA per window, alternating between the two HWDGE rings
    (sync=SP, scalar=ACT). bf16 halves write-side HBM traffic; tolerance is
    2e-2 and bf16 round-off is ~2e-3.
  * Host applies the recorded inverse permutation to scatter staged rows into
    the final [4, 204800, 64] f32 output (host-side unshard).
"""

import sys

import numpy as np

for _p in ("/opt/trn_rl_repo",):
    if _p not in sys.path:
        sys.path.insert(0, _p)

F = 4
V = 1_000_000
D = 64
BL = 204_800
PERM = (2, 0, 3, 1)

N_CORES = 8
P = 128
ROWS_TOTAL = F * BL                   # 819,200 lookups
SHARD = F * V // N_CORES              # 500,000 table rows per core
WIN = 32_768                          # int16-addressable window
N_FULL_WIN = SHARD // WIN             # 15 full windows
LAST_WIN_ROWS = SHARD - N_FULL_WIN * WIN  # 8,480
N_WIN = N_FULL_WIN + 1                # 16 windows per core

GRANULE = 1024          # idxs per dma_gather (single-packet: 64 desc/engine)
N_SWDGE_QUEUES = 4
# The SWDGE descriptor-ring carveout is sized by the COMPILER flag
# --internal-dynamic-dma-scratch-size-per-partition (default 16384 B =>
# 128 descs per queue/direction ring => only ~2 gathers in flight per
# queue; the gather ucode's await_space then block-waits on the previous
# DMA's completion sem, which measured ~9us per 4-gather round and capped
# every design at ~290us). 65536 quadruples the ring: ~8 gathers in
# flight per queue, enough to keep the 16 SDMA engines saturated.
DMA_SCRATCH = 65536
NBUF = 6                # f32 window tiles in flight (~13KB/partition each)
NBUF_BF = 4             # bf16 writeback tiles (~6.5KB/partition each)


def _ensure_dma_ring_flag():
    from concourse.compiler_utils import get_compiler_flags, set_compiler_flags

    pref = "--internal-dynamic-dma-scratch-size-per-partition"
    want = f"{pref}={DMA_SCRATCH}"
    flags = get_compiler_flags()
    if want not in flags:
        flags = [f for f in flags if not f.startswith(pref)]
        set_compiler_flags(flags + [want])

WIN_ROWS = [WIN] * N_FULL_WIN + [LAST_WIN_ROWS]


def build_nc(plan):
    """Per-core SPMD raw-bass program from route()'s granule plan."""
    import concourse.bacc as bacc
    import concourse.mybir as mybir

    _ensure_dma_ring_flag()
    gplan, pcols, scols, idx_cols = plan
    rowsper = [2 * pcols[w] + scols[w] for w in range(N_WIN)]
    stage_rows = sum(P * r for r in rowsper)
    stage_off = np.cumsum([0] + [P * r for r in rowsper]).tolist()
    wcols16 = [(pcols[w] + scols[w]) * 8 for w in range(N_WIN)]
    idx_off = np.cumsum([0] + wcols16).tolist()
    p_off16 = [idx_off[w] for w in range(N_WIN)]
    s_off16 = [idx_off[w] + pcols[w] * 8 for w in range(N_WIN)]
    assert idx_off[-1] == idx_cols
    by_window = {}
    n_regs = 0
    for kind, w, g, n, reg_i in gplan:
        by_window.setdefault(w, []).append((kind, g, n, reg_i))
        if reg_i is not None:
            n_regs = max(n_regs, reg_i + 1)
        assert n <= GRANULE

    nc = bacc.Bacc(
        None,
        num_swdge_queues=N_SWDGE_QUEUES,
        dynamic_dma_scratch_size=DMA_SCRATCH,
    )
    tabs = [
        nc.declare_dram_parameter(
            f"tab{w}", [WIN_ROWS[w], D], mybir.dt.float32, isOutput=False
        )
        for w in range(N_WIN)
    ]
    idx_in = nc.declare_dram_parameter(
        "idx", [P, idx_cols], mybir.dt.int16, isOutput=False
    )
    cnt_in = nc.declare_dram_parameter(
        "cnt", [1, max(n_regs, 1)], mybir.dt.int32, isOutput=False
    )
    out = nc.declare_dram_parameter(
        "out", [stage_rows, D], mybir.dt.bfloat16, isOutput=True
    )

    regs = [
        nc.alloc_register(mybir.EngineType.Pool, f"cnt_reg{i}")
        for i in range(n_regs)
    ]
    idx_tile = nc.alloc_sbuf_tensor("idxt", [P, idx_cols], mybir.dt.int16)
    cnt_tile = nc.alloc_sbuf_tensor("cntt", [1, max(n_regs, 1)], mybir.dt.int32)
    dummy_dst = nc.alloc_sbuf_tensor("dmyd", [P, D], mybir.dt.float32)
    maxrp = max(rowsper)
    datas = [
        nc.alloc_sbuf_tensor(f"data{i}", [P, maxrp * D], mybir.dt.float32)
        for i in range(NBUF)
    ]
    bfs = [
        nc.alloc_sbuf_tensor(f"bf{i}", [P, maxrp * D], mybir.dt.bfloat16)
        for i in range(NBUF_BF)
    ]

    dsem = nc.alloc_semaphore("dsem")        # first idx slice landed
    cnt_sem = nc.alloc_semaphore("cntsem")   # cnt tensor landed
    isem = nc.alloc_semaphore("isem")        # idx window loads (16 each)
    csem = nc.alloc_semaphore("csem")        # casts completed (1 each)
    wsem_s = nc.alloc_semaphore("wsem_s")    # sync-ring writebacks (16 each)
    wsem_a = nc.alloc_semaphore("wsem_a")    # scalar-ring writebacks (16 each)
    # one DMA-completion sem per SWDGE queue (a sem is HW-locked to a single
    # queue); granules round-robin the queues and the per-window completion
    # condition is the 4 cumulative per-queue counts after that window.
    qsems = [nc.alloc_semaphore(f"qsem{q}") for q in range(N_SWDGE_QUEUES)]

    # ---- sync (SP) engine: input loads, then even-window writebacks ----
    nc.sync.dma_start(out=idx_tile[:, 0:8], in_=idx_in[:, 0:8]).then_inc(dsem, 16)
    nc.sync.dma_start(out=cnt_tile[:], in_=cnt_in[:]).then_inc(cnt_sem, 16)
    for w in range(N_WIN):
        nc.sync.dma_start(
            out=idx_tile[:, idx_off[w] : idx_off[w + 1]],
            in_=idx_in[:, idx_off[w] : idx_off[w + 1]],
        ).then_inc(isem, 16)

    # ---- Pool engine: warmup gather (pulls the ucode library reload to the
    # top of the stream, overlapping the input loads), count-register
    # preloads, then the gather stream (runs ahead of casts by NBUF windows).
    nc.gpsimd.wait_ge(dsem, 16)
    nc.gpsimd.dma_gather(
        dummy_dst[:].rearrange("p (c d) -> p c d", d=D),
        tabs[0][:],
        idx_tile[:, 0:1],
        16,
        16,
        D,
        single_packet=True,
        queue_num=0,
    ).then_inc(qsems[0], 16)
    nc.gpsimd.wait_ge(cnt_sem, 16)
    for i in range(n_regs):
        nc.gpsimd.reg_load(regs[i], cnt_tile[0:1, i : i + 1])
    # Each gather occupies Q7 core pair q (queue q) for ~8.5us of descriptor
    # emission; round-robining granules over all 4 queues overlaps 4 pairs
    # for an effective ~2.2us/granule. 4 queues is the ucode max
    # (MAX_SWDGE_QUEUES); a single queue measures 3x slower (874us).
    qcum = [16] + [0] * (N_SWDGE_QUEUES - 1)   # warmup bumped qsems[0]
    win_qcum = []                              # per-window cumulative targets
    g_idx = 0
    for w in range(N_WIN):
        nc.gpsimd.wait_ge(isem, 16 * (w + 1))
        if w >= NBUF:
            nc.gpsimd.wait_ge(csem, w - NBUF + 1)
        data = datas[w % NBUF]
        for kind, g, n, reg_i in by_window[w]:
            q = g_idx % N_SWDGE_QUEUES
            nreg = regs[reg_i] if reg_i is not None else n
            if kind == "p":
                # gap-1 pairs: overlapping table view [rows-1, 128] with row
                # stride 64 f32 — one 512B descriptor covers rows r, r+1
                src = tabs[w][:].rearrange("(a b) d -> a (b d)", b=2)
                src.ap[0] = [D, WIN_ROWS[w] - 1]
                f0 = g * (GRANULE // P) * 2 * D
                ncols = (n + P - 1) // P
                dst = data[:, f0 : f0 + ncols * 2 * D].rearrange(
                    "p (c d2) -> p c d2", d2=2 * D
                )
                c0 = p_off16[w] + g * (GRANULE // 16)
                nc.gpsimd.dma_gather(
                    dst, src, idx_tile[:, c0 : c0 + (n + 15) // 16],
                    n, nreg, 2 * D, elem_step=D,
                    single_packet=True, queue_num=q,
                ).then_inc(qsems[q], 16)
            else:
                f0 = (2 * pcols[w] + g * (GRANULE // P)) * D
                ncols = (n + P - 1) // P
                c0 = s_off16[w] + g * (GRANULE // 16)
                nc.gpsimd.dma_gather(
                    data[:, f0 : f0 + ncols * D].rearrange(
                        "p (c d) -> p c d", d=D
                    ),
                    tabs[w][:],
                    idx_tile[:, c0 : c0 + (n + 15) // 16],
                    n, nreg, D, single_packet=True, queue_num=q,
                ).then_inc(qsems[q], 16)
            qcum[q] += 16
            g_idx += 1
        win_qcum.append(list(qcum))

    # ---- DVE: per-window f32->bf16 cast once the window's gathers land ----
    prev = [0] * N_SWDGE_QUEUES
    for w in range(N_WIN):
        for q in range(N_SWDGE_QUEUES):
            if win_qcum[w][q] > prev[q]:
                nc.vector.wait_ge(qsems[q], win_qcum[w][q])
        prev = win_qcum[w]
        if w >= NBUF_BF:
            pw = w - NBUF_BF
            wsem = wsem_s if pw % 2 == 0 else wsem_a
            nc.vector.wait_ge(wsem, 16 * (pw // 2 + 1))
        nbytes = rowsper[w] * D
        nc.vector.tensor_scalar_mul(
            bfs[w % NBUF_BF][:, :nbytes], datas[w % NBUF][:, :nbytes], 1.0
        ).then_inc(csem, 1)

    # ---- writebacks: even windows on the sync (SP) HWDGE ring, odd on the
    # scalar (ACT) ring ----
    for w in range(N_WIN):
        eng = nc.sync if w % 2 == 0 else nc.scalar
        wsem = wsem_s if w % 2 == 0 else wsem_a
        eng.wait_ge(csem, w + 1)
        win_ap = out[stage_off[w] : stage_off[w + 1], :].rearrange(
            "(p c) d -> p (c d)", p=P
        )
        eng.dma_start(
            out=win_ap[:], in_=bfs[w % NBUF_BF][:, : rowsper[w] * D]
        ).then_inc(wsem, 16)
    nc.sync.wait_ge(wsem_s, 16 * ((N_WIN + 1) // 2))
    nc.scalar.wait_ge(wsem_a, 16 * (N_WIN // 2))

    nc.compile()
    return nc


def _bucket_ordinals(mask, dgrp, n_groups):
    m = np.flatnonzero(mask)
    cnts = np.bincount(dgrp[m], minlength=n_groups).astype(np.int64)
    off = np.zeros(n_groups + 1, np.int64)
    np.cumsum(cnts, out=off[1:])
    ords = np.zeros(len(dgrp), np.int64)
    ords[m] = np.arange(m.size) - np.repeat(off[:-1], cnts)
    return ords, cnts


def route(indices):
    """Host-side routing with gap-1 pair packing.

    Distinct rows per (core, window) bucket split into PAIRS (rows r, r+1
    both present: one 512B descriptor via an overlapping [rows-1, 128]
    table view with row stride 64, idx = r) and SINGLES (256B row gather).
    """
    idx = np.asarray(indices)
    perm = np.asarray(PERM)
    glob = (idx[perm].astype(np.int64) + (perm * V)[:, None]).reshape(-1)

    core = glob // SHARD
    local = glob - core * SHARD
    win = local // WIN
    wlocal = local - win * WIN

    group = core * N_WIN + win
    order = np.lexsort((wlocal, group))
    g_sorted = group[order]
    w_sorted = wlocal[order]

    first = np.ones(glob.size, bool)
    first[1:] = (g_sorted[1:] != g_sorted[:-1]) | (w_sorted[1:] != w_sorted[:-1])
    fc = np.cumsum(first) - 1

    n_groups = N_CORES * N_WIN
    didx = np.flatnonzero(first)
    dgrp = g_sorted[didx]
    drow = w_sorted[didx]
    n_d = didx.size

    # greedy gap-1 pairing within each bucket
    edge = np.zeros(n_d, bool)
    edge[:-1] = (dgrp[1:] == dgrp[:-1]) & (drow[1:] == drow[:-1] + 1)
    prev_edge = np.zeros(n_d, bool)
    prev_edge[1:] = edge[:-1]
    in_chain = edge | prev_edge
    run_start = in_chain & ~prev_edge
    starts = np.flatnonzero(run_start)
    chain = np.flatnonzero(in_chain)
    runpos = np.zeros(n_d, np.int64)
    if starts.size:
        run_of = np.cumsum(run_start)[in_chain] - 1
        runpos[in_chain] = chain - starts[run_of]
    pair_first = in_chain & (runpos % 2 == 0) & edge
    pair_second = np.zeros(n_d, bool)
    pair_second[1:] = pair_first[:-1]
    single = ~pair_first & ~pair_second

    p_ord, p_cnt = _bucket_ordinals(pair_first, dgrp, n_groups)
    # cap pairs at one full granule per window: demote overflow pairs (rare)
    # back to singles so no runt pair granules fragment the queue rotation
    over = pair_first & (p_ord >= GRANULE)
    if over.any():
        sec = np.zeros(n_d, bool)
        sec[1:] = over[:-1]
        pair_first = pair_first & ~over
        pair_second = pair_second & ~sec
        single = ~pair_first & ~pair_second
        p_ord, p_cnt = _bucket_ordinals(pair_first, dgrp, n_groups)
    s_ord, s_cnt = _bucket_ordinals(single, dgrp, n_groups)
    p_cnt = p_cnt.reshape(N_CORES, N_WIN)
    s_cnt = s_cnt.reshape(N_CORES, N_WIN)

    rup = lambda a: (np.maximum(a, 1) + P - 1) // P * P
    pads_p = rup(p_cnt.max(axis=0)).astype(np.int64)
    pads_s = rup(s_cnt.max(axis=0)).astype(np.int64)
    full_s = (s_cnt.min(axis=0) // GRANULE).astype(np.int64)

    pcols, scols = pads_p // P, pads_s // P
    rowsper = 2 * pcols + scols
    stage_off = np.cumsum([0] + (P * rowsper).tolist())[:-1]

    wcols16 = (pads_p + pads_s) // 16
    idx_off16 = np.cumsum([0] + wcols16.tolist())[:-1]
    idx_cols = int(wcols16.sum())
    p_off16 = idx_off16
    s_off16 = idx_off16 + pads_p // 16

    wd = dgrp % N_WIN
    cd = dgrp // N_WIN
    idx16 = np.full((N_CORES, 16, idx_cols), -1, dtype=np.int16)
    m = pair_first
    idx16[cd[m], p_ord[m] % 16, p_off16[wd[m]] + p_ord[m] // 16] = drow[m].astype(
        np.int16
    )
    m = single
    idx16[cd[m], s_ord[m] % 16, s_off16[wd[m]] + s_ord[m] // 16] = drow[m].astype(
        np.int16
    )

    src_d = np.zeros(n_d, np.int64)
    b = stage_off[wd]
    rp = rowsper[wd]
    m = pair_first
    psub, pr = p_ord[m] // GRANULE, p_ord[m] % GRANULE
    src_d[m] = (
        b[m] + (pr % P) * rp[m] + 2 * (psub * (GRANULE // P) + pr // P)
    )
    m = single
    sub, r = s_ord[m] // GRANULE, s_ord[m] % GRANULE
    src_d[m] = (
        b[m] + (r % P) * rp[m] + 2 * pcols[wd[m]] + sub * (GRANULE // P) + r // P
    )
    src_d[pair_second] = src_d[np.flatnonzero(pair_second) - 1] + 1

    chk = np.zeros(n_d, np.int64)
    chk[pair_first] = drow[pair_first]
    chk[pair_second] = chk[np.flatnonzero(pair_second) - 1] + 1
    chk[single] = drow[single]
    assert np.array_equal(chk, drow), "pair packing reconstruction failed"

    src_rows = src_d[fc]

    reg_specs = []   # (kind, w, g)
    gplan = []   # (kind, w, g, num_idxs, reg or None)
    for w in range(N_WIN):
        ngp = (int(pads_p[w]) + GRANULE - 1) // GRANULE
        for g in range(ngp):
            n = min(GRANULE, int(pads_p[w]) - g * GRANULE)
            reg_specs.append(("p", w, g))
            gplan.append(("p", w, g, n, len(reg_specs) - 1))
        ng = (int(pads_s[w]) + GRANULE - 1) // GRANULE
        for g in range(ng):
            n = min(GRANULE, int(pads_s[w]) - g * GRANULE)
            if g < full_s[w]:
                gplan.append(("s", w, g, n, None))
            else:
                reg_specs.append(("s", w, g))
                gplan.append(("s", w, g, n, len(reg_specs) - 1))
    n_regs = len(reg_specs)

    cnts = np.zeros((N_CORES, max(n_regs, 1)), np.int32)
    for i, (kind, w, g) in enumerate(reg_specs):
        cnt_src = p_cnt if kind == "p" else s_cnt
        off = p_off16 if kind == "p" else s_off16
        c = np.clip(cnt_src[:, w] - g * GRANULE, 0, GRANULE).astype(np.int32)
        empty = np.flatnonzero(c == 0)
        if empty.size:
            idx16[empty, 0, off[w] + g * (GRANULE // 16)] = 0
            c[empty] = 1
        cnts[:, i] = c

    idx_inputs = np.ascontiguousarray(np.tile(idx16, (1, 8, 1)))
    c_of = g_sorted // N_WIN
    plan = (
        tuple(gplan),
        tuple(int(x) for x in pcols),
        tuple(int(x) for x in scols),
        idx_cols,
    )
    return idx_inputs, order, src_rows, c_of, cnts, plan


_NC_CACHE = {}


def _get_nc(plan):
    if plan not in _NC_CACHE:
        _NC_CACHE[plan] = build_nc(plan)
    return _NC_CACHE[plan]


def run_sharded(indices, tables, trace=False, **spmd_kwargs):
    """Run the SPMD kernel on 8 cores; returns (full_output, BassKernelResults)."""
    from concourse import bass_utils

    tables_flat = np.asarray(tables, dtype=np.float32).reshape(F * V, D)
    idx_inputs, dst_rows, src_rows, core_of, cnts, plan = route(indices)

    in_maps = []
    for c in range(N_CORES):
        m = {"idx": idx_inputs[c], "cnt": cnts[c : c + 1]}
        shard = tables_flat[c * SHARD : (c + 1) * SHARD]
        r0 = 0
        for w in range(N_WIN):
            m[f"tab{w}"] = shard[r0 : r0 + WIN_ROWS[w]]
            r0 += WIN_ROWS[w]
        in_maps.append(m)

    nc = _get_nc(plan)
    res = bass_utils.run_bass_kernel_spmd(
        nc, in_maps, list(range(N_CORES)), trace=trace, **spmd_kwargs
    )

    out_flat = np.empty((ROWS_TOTAL, D), dtype=np.float32)
    for c in range(N_CORES):
        sel = core_of == c
        staged = np.asarray(res.results[c]["out"]).astype(np.float32)
        out_flat[dst_rows[sel]] = staged[src_rows[sel]]
    return out_flat.reshape(F, BL, D), res


def kernel(indices, tables):
    out, _ = run_sharded(indices, tables, trace=False)
    return out


# revision 40
# speedup vs baseline: 1.0456x; 1.0456x over previous
"""Trainium2 Bass kernel: multi-table embedding gather (pooling=NONE).

Reference computation (hardcoded shapes):
    indices: [F=4, BL=204800] int   (values in [0, V))
    tables:  [F=4, V=1e6, D=64] f32
    out[f]   = tables[PERM[f]][indices[PERM[f]]]   -> [4, 204800, 64] f32
    PERM = [2, 0, 3, 1]

Strategy (model/table-parallel, per the sharding hint):
  * Fold the table permutation into global row ids g = PERM[f]*V + idx over a
    flat [4M, 64] table; shard row-wise across 8 cores (500,000 rows each).
  * Host routes every lookup to its owning core, bucketing by 32,768-row
    window so the gather uses the int16 `dma_gather` SWDGE ucode with
    1024-idx single-packet sub-gathers (64 descriptors/engine, the packet
    ceiling; multi-packet and prepare_only modes are ~10-100x slower per
    descriptor on the Q7).
  * RAW bass pipeline (no TileContext): Tile tracks each SWDGE DMA on one of
    8 DMASW semaphore lanes, which made every gather wait for the FULL
    completion of the gather 8 before it; the engines ping-ponged between
    ~4.5us of full-rate drain and ~5us of idle (measured 290us). Here each
    window's gathers share one explicit semaphore (+16/gather), so the Pool
    engine runs ~NBUF windows ahead and the SDMA engines stay saturated.
  * Within a window, granules guaranteed full on every core use an immediate
    count; the partial tail granule reads a preloaded count register.
  * Gathered f32 rows are cast to bf16 on the DVE and written back with one
    contiguous HWDGE DMA per window, alternating between the two HWDGE rings
    (sync=SP, scalar=ACT). bf16 halves write-side HBM traffic; tolerance is
    2e-2 and bf16 round-off is ~2e-3.
  * Host applies the recorded inverse permutation to scatter staged rows into
    the final [4, 204800, 64] f32 output (host-side unshard).
"""

import sys

import numpy as np

for _p in ("/opt/trn_rl_repo",):
    if _p not in sys.path:
        sys.path.insert(0, _p)

F = 4
V = 1_000_000
D = 64
BL = 204_800
PERM = (2, 0, 3, 1)

N_CORES = 8
P = 128
ROWS_TOTAL = F * BL                   # 819,200 lookups
SHARD = F * V // N_CORES              # 500,000 table rows per core
WIN = 32_768                          # int16-addressable window
N_FULL_WIN = SHARD // WIN             # 15 full windows
LAST_WIN_ROWS = SHARD - N_FULL_WIN * WIN  # 8,480
N_WIN = N_FULL_WIN + 1                # 16 windows per core

GRANULE = 1024          # idxs per dma_gather (single-packet: 64 desc/engine)
N_SWDGE_QUEUES = 4
# The SWDGE descriptor-ring carveout is sized by the COMPILER flag
# --internal-dynamic-dma-scratch-size-per-partition (default 16384 B =>
# 128 descs per queue/direction ring => only ~2 gathers in flight per
# queue; the gather ucode's await_space then block-waits on the previous
# DMA's completion sem, which measured ~9us per 4-gather round and capped
# every design at ~290us). 65536 quadruples the ring: ~8 gathers in
# flight per queue, enough to keep the 16 SDMA engines saturated.
DMA_SCRATCH = 65536
NBUF = 7                # f32 window tiles in flight (~13KB/partition each)
NBUF_BF = 4             # bf16 writeback tiles (~6.5KB/partition each)


def _ensure_dma_ring_flag():
    from concourse.compiler_utils import get_compiler_flags, set_compiler_flags

    pref = "--internal-dynamic-dma-scratch-size-per-partition"
    want = f"{pref}={DMA_SCRATCH}"
    flags = get_compiler_flags()
    if want not in flags:
        flags = [f for f in flags if not f.startswith(pref)]
        set_compiler_flags(flags + [want])

WIN_ROWS = [WIN] * N_FULL_WIN + [LAST_WIN_ROWS]


def build_nc(plan):
    """Per-core SPMD raw-bass program from route()'s granule plan."""
    import concourse.bacc as bacc
    import concourse.mybir as mybir

    _ensure_dma_ring_flag()
    gplan, pcols, scols, idx_cols = plan
    rowsper = [2 * pcols[w] + scols[w] for w in range(N_WIN)]
    stage_rows = sum(P * r for r in rowsper)
    stage_off = np.cumsum([0] + [P * r for r in rowsper]).tolist()
    wcols16 = [(pcols[w] + scols[w]) * 8 for w in range(N_WIN)]
    idx_off = np.cumsum([0] + wcols16).tolist()
    p_off16 = [idx_off[w] for w in range(N_WIN)]
    s_off16 = [idx_off[w] + pcols[w] * 8 for w in range(N_WIN)]
    assert idx_off[-1] == idx_cols
    by_window = {}
    n_regs = 0
    for kind, w, g, n, reg_i in gplan:
        by_window.setdefault(w, []).append((kind, g, n, reg_i))
        if reg_i is not None:
            n_regs = max(n_regs, reg_i + 1)
        assert n <= GRANULE

    nc = bacc.Bacc(
        None,
        num_swdge_queues=N_SWDGE_QUEUES,
        dynamic_dma_scratch_size=DMA_SCRATCH,
    )
    tabs = [
        nc.declare_dram_parameter(
            f"tab{w}", [WIN_ROWS[w], D], mybir.dt.float32, isOutput=False
        )
        for w in range(N_WIN)
    ]
    idx_in = nc.declare_dram_parameter(
        "idx", [P, idx_cols], mybir.dt.int16, isOutput=False
    )
    cnt_in = nc.declare_dram_parameter(
        "cnt", [1, max(n_regs, 1)], mybir.dt.int32, isOutput=False
    )
    out = nc.declare_dram_parameter(
        "out", [stage_rows, D], mybir.dt.bfloat16, isOutput=True
    )

    regs = [
        nc.alloc_register(mybir.EngineType.Pool, f"cnt_reg{i}")
        for i in range(n_regs)
    ]
    idx_tile = nc.alloc_sbuf_tensor("idxt", [P, idx_cols], mybir.dt.int16)
    cnt_tile = nc.alloc_sbuf_tensor("cntt", [1, max(n_regs, 1)], mybir.dt.int32)
    dummy_dst = nc.alloc_sbuf_tensor("dmyd", [P, D], mybir.dt.float32)
    maxrp = max(rowsper)
    datas = [
        nc.alloc_sbuf_tensor(f"data{i}", [P, maxrp * D], mybir.dt.float32)
        for i in range(NBUF)
    ]
    bfs = [
        nc.alloc_sbuf_tensor(f"bf{i}", [P, maxrp * D], mybir.dt.bfloat16)
        for i in range(NBUF_BF)
    ]

    dsem = nc.alloc_semaphore("dsem")        # first idx slice landed
    cnt_sem = nc.alloc_semaphore("cntsem")   # cnt tensor landed
    isem = nc.alloc_semaphore("isem")        # idx window loads (16 each)
    csem = nc.alloc_semaphore("csem")        # casts completed (1 each)
    wsem_s = nc.alloc_semaphore("wsem_s")    # sync-ring writebacks (16 each)
    wsem_a = nc.alloc_semaphore("wsem_a")    # scalar-ring writebacks (16 each)
    # one DMA-completion sem per SWDGE queue (a sem is HW-locked to a single
    # queue); granules round-robin the queues and the per-window completion
    # condition is the 4 cumulative per-queue counts after that window.
    qsems = [nc.alloc_semaphore(f"qsem{q}") for q in range(N_SWDGE_QUEUES)]

    # ---- sync (SP) engine: input loads, then even-window writebacks ----
    nc.sync.dma_start(out=idx_tile[:, 0:8], in_=idx_in[:, 0:8]).then_inc(dsem, 16)
    nc.sync.dma_start(out=cnt_tile[:], in_=cnt_in[:]).then_inc(cnt_sem, 16)
    for w in range(N_WIN):
        nc.sync.dma_start(
            out=idx_tile[:, idx_off[w] : idx_off[w + 1]],
            in_=idx_in[:, idx_off[w] : idx_off[w + 1]],
        ).then_inc(isem, 16)

    # ---- Pool engine: warmup gather (pulls the ucode library reload to the
    # top of the stream, overlapping the input loads), count-register
    # preloads, then the gather stream (runs ahead of casts by NBUF windows).
    nc.gpsimd.wait_ge(dsem, 16)
    nc.gpsimd.dma_gather(
        dummy_dst[:].rearrange("p (c d) -> p c d", d=D),
        tabs[0][:],
        idx_tile[:, 0:1],
        16,
        16,
        D,
        single_packet=True,
        queue_num=0,
    ).then_inc(qsems[0], 16)
    nc.gpsimd.wait_ge(cnt_sem, 16)
    for i in range(n_regs):
        nc.gpsimd.reg_load(regs[i], cnt_tile[0:1, i : i + 1])
    # Each gather occupies Q7 core pair q (queue q) for ~8.5us of descriptor
    # emission; round-robining granules over all 4 queues overlaps 4 pairs
    # for an effective ~2.2us/granule. 4 queues is the ucode max
    # (MAX_SWDGE_QUEUES); a single queue measures 3x slower (874us).
    qcum = [16] + [0] * (N_SWDGE_QUEUES - 1)   # warmup bumped qsems[0]
    win_qcum = []                              # per-window cumulative targets
    g_idx = 0
    for w in range(N_WIN):
        nc.gpsimd.wait_ge(isem, 16 * (w + 1))
        if w >= NBUF:
            nc.gpsimd.wait_ge(csem, w - NBUF + 1)
        data = datas[w % NBUF]
        for kind, g, n, reg_i in by_window[w]:
            q = g_idx % N_SWDGE_QUEUES
            nreg = regs[reg_i] if reg_i is not None else n
            if kind == "p":
                # gap-1 pairs: overlapping table view [rows-1, 128] with row
                # stride 64 f32 — one 512B descriptor covers rows r, r+1
                src = tabs[w][:].rearrange("(a b) d -> a (b d)", b=2)
                src.ap[0] = [D, WIN_ROWS[w] - 1]
                f0 = g * (GRANULE // P) * 2 * D
                ncols = (n + P - 1) // P
                dst = data[:, f0 : f0 + ncols * 2 * D].rearrange(
                    "p (c d2) -> p c d2", d2=2 * D
                )
                c0 = p_off16[w] + g * (GRANULE // 16)
                nc.gpsimd.dma_gather(
                    dst, src, idx_tile[:, c0 : c0 + (n + 15) // 16],
                    n, nreg, 2 * D, elem_step=D,
                    single_packet=True, queue_num=q,
                ).then_inc(qsems[q], 16)
            else:
                f0 = (2 * pcols[w] + g * (GRANULE // P)) * D
                ncols = (n + P - 1) // P
                c0 = s_off16[w] + g * (GRANULE // 16)
                nc.gpsimd.dma_gather(
                    data[:, f0 : f0 + ncols * D].rearrange(
                        "p (c d) -> p c d", d=D
                    ),
                    tabs[w][:],
                    idx_tile[:, c0 : c0 + (n + 15) // 16],
                    n, nreg, D, single_packet=True, queue_num=q,
                ).then_inc(qsems[q], 16)
            qcum[q] += 16
            g_idx += 1
        win_qcum.append(list(qcum))

    # ---- DVE: per-window f32->bf16 cast once the window's gathers land ----
    prev = [0] * N_SWDGE_QUEUES
    for w in range(N_WIN):
        for q in range(N_SWDGE_QUEUES):
            if win_qcum[w][q] > prev[q]:
                nc.vector.wait_ge(qsems[q], win_qcum[w][q])
        prev = win_qcum[w]
        if w >= NBUF_BF:
            pw = w - NBUF_BF
            wsem = wsem_s if pw % 2 == 0 else wsem_a
            nc.vector.wait_ge(wsem, 16 * (pw // 2 + 1))
        nbytes = rowsper[w] * D
        nc.vector.tensor_scalar_mul(
            bfs[w % NBUF_BF][:, :nbytes], datas[w % NBUF][:, :nbytes], 1.0
        ).then_inc(csem, 1)

    # ---- writebacks: even windows on the sync (SP) HWDGE ring, odd on the
    # scalar (ACT) ring ----
    for w in range(N_WIN):
        eng = nc.sync if w % 2 == 0 else nc.scalar
        wsem = wsem_s if w % 2 == 0 else wsem_a
        eng.wait_ge(csem, w + 1)
        win_ap = out[stage_off[w] : stage_off[w + 1], :].rearrange(
            "(p c) d -> p (c d)", p=P
        )
        eng.dma_start(
            out=win_ap[:], in_=bfs[w % NBUF_BF][:, : rowsper[w] * D]
        ).then_inc(wsem, 16)
    nc.sync.wait_ge(wsem_s, 16 * ((N_WIN + 1) // 2))
    nc.scalar.wait_ge(wsem_a, 16 * (N_WIN // 2))

    nc.compile()
    return nc


def _bucket_ordinals(mask, dgrp, n_groups):
    m = np.flatnonzero(mask)
    cnts = np.bincount(dgrp[m], minlength=n_groups).astype(np.int64)
    off = np.zeros(n_groups + 1, np.int64)
    np.cumsum(cnts, out=off[1:])
    ords = np.zeros(len(dgrp), np.int64)
    ords[m] = np.arange(m.size) - np.repeat(off[:-1], cnts)
    return ords, cnts


def route(indices):
    """Host-side routing with gap-1 pair packing.

    Distinct rows per (core, window) bucket split into PAIRS (rows r, r+1
    both present: one 512B descriptor via an overlapping [rows-1, 128]
    table view with row stride 64, idx = r) and SINGLES (256B row gather).
    """
    idx = np.asarray(indices)
    perm = np.asarray(PERM)
    glob = (idx[perm].astype(np.int64) + (perm * V)[:, None]).reshape(-1)

    core = glob // SHARD
    local = glob - core * SHARD
    win = local // WIN
    wlocal = local - win * WIN

    group = core * N_WIN + win
    order = np.lexsort((wlocal, group))
    g_sorted = group[order]
    w_sorted = wlocal[order]

    first = np.ones(glob.size, bool)
    first[1:] = (g_sorted[1:] != g_sorted[:-1]) | (w_sorted[1:] != w_sorted[:-1])
    fc = np.cumsum(first) - 1

    n_groups = N_CORES * N_WIN
    didx = np.flatnonzero(first)
    dgrp = g_sorted[didx]
    drow = w_sorted[didx]
    n_d = didx.size

    # greedy gap-1 pairing within each bucket
    edge = np.zeros(n_d, bool)
    edge[:-1] = (dgrp[1:] == dgrp[:-1]) & (drow[1:] == drow[:-1] + 1)
    prev_edge = np.zeros(n_d, bool)
    prev_edge[1:] = edge[:-1]
    in_chain = edge | prev_edge
    run_start = in_chain & ~prev_edge
    starts = np.flatnonzero(run_start)
    chain = np.flatnonzero(in_chain)
    runpos = np.zeros(n_d, np.int64)
    if starts.size:
        run_of = np.cumsum(run_start)[in_chain] - 1
        runpos[in_chain] = chain - starts[run_of]
    pair_first = in_chain & (runpos % 2 == 0) & edge
    pair_second = np.zeros(n_d, bool)
    pair_second[1:] = pair_first[:-1]
    single = ~pair_first & ~pair_second

    p_ord, p_cnt = _bucket_ordinals(pair_first, dgrp, n_groups)
    # cap pairs at one full granule per window: demote overflow pairs (rare)
    # back to singles so no runt pair granules fragment the queue rotation
    over = pair_first & (p_ord >= GRANULE)
    if over.any():
        sec = np.zeros(n_d, bool)
        sec[1:] = over[:-1]
        pair_first = pair_first & ~over
        pair_second = pair_second & ~sec
        single = ~pair_first & ~pair_second
        p_ord, p_cnt = _bucket_ordinals(pair_first, dgrp, n_groups)
    s_ord, s_cnt = _bucket_ordinals(single, dgrp, n_groups)
    p_cnt = p_cnt.reshape(N_CORES, N_WIN)
    s_cnt = s_cnt.reshape(N_CORES, N_WIN)

    rup = lambda a: (np.maximum(a, 1) + P - 1) // P * P
    pads_p = rup(p_cnt.max(axis=0)).astype(np.int64)
    pads_s = rup(s_cnt.max(axis=0)).astype(np.int64)
    full_s = (s_cnt.min(axis=0) // GRANULE).astype(np.int64)

    pcols, scols = pads_p // P, pads_s // P
    rowsper = 2 * pcols + scols
    stage_off = np.cumsum([0] + (P * rowsper).tolist())[:-1]

    wcols16 = (pads_p + pads_s) // 16
    idx_off16 = np.cumsum([0] + wcols16.tolist())[:-1]
    idx_cols = int(wcols16.sum())
    p_off16 = idx_off16
    s_off16 = idx_off16 + pads_p // 16

    wd = dgrp % N_WIN
    cd = dgrp // N_WIN
    idx16 = np.full((N_CORES, 16, idx_cols), -1, dtype=np.int16)
    m = pair_first
    idx16[cd[m], p_ord[m] % 16, p_off16[wd[m]] + p_ord[m] // 16] = drow[m].astype(
        np.int16
    )
    m = single
    idx16[cd[m], s_ord[m] % 16, s_off16[wd[m]] + s_ord[m] // 16] = drow[m].astype(
        np.int16
    )
    # pair granules use immediate counts: fill padding with idx 0 (valid)
    for w in range(N_WIN):
        reg = idx16[:, :, p_off16[w] : p_off16[w] + int(pads_p[w]) // 16]
        reg[reg == -1] = 0

    src_d = np.zeros(n_d, np.int64)
    b = stage_off[wd]
    rp = rowsper[wd]
    m = pair_first
    psub, pr = p_ord[m] // GRANULE, p_ord[m] % GRANULE
    src_d[m] = (
        b[m] + (pr % P) * rp[m] + 2 * (psub * (GRANULE // P) + pr // P)
    )
    m = single
    sub, r = s_ord[m] // GRANULE, s_ord[m] % GRANULE
    src_d[m] = (
        b[m] + (r % P) * rp[m] + 2 * pcols[wd[m]] + sub * (GRANULE // P) + r // P
    )
    src_d[pair_second] = src_d[np.flatnonzero(pair_second) - 1] + 1

    chk = np.zeros(n_d, np.int64)
    chk[pair_first] = drow[pair_first]
    chk[pair_second] = chk[np.flatnonzero(pair_second) - 1] + 1
    chk[single] = drow[single]
    assert np.array_equal(chk, drow), "pair packing reconstruction failed"

    src_rows = src_d[fc]

    reg_specs = []
    gplan = []   # (kind, w, s, num_idxs, reg or None)
    for w in range(N_WIN):
        ngp = (int(pads_p[w]) + GRANULE - 1) // GRANULE
        for g in range(ngp):
            n = min(GRANULE, int(pads_p[w]) - g * GRANULE)
            gplan.append(("p", w, g, n, None))
        ng = (int(pads_s[w]) + GRANULE - 1) // GRANULE
        for g in range(ng):
            n = min(GRANULE, int(pads_s[w]) - g * GRANULE)
            if g < full_s[w]:
                gplan.append(("s", w, g, n, None))
            else:
                reg_specs.append((w, g))
                gplan.append(("s", w, g, n, len(reg_specs) - 1))
    n_regs = len(reg_specs)

    cnts = np.zeros((N_CORES, max(n_regs, 1)), np.int32)
    for i, (w, g) in enumerate(reg_specs):
        c = np.clip(s_cnt[:, w] - g * GRANULE, 0, GRANULE).astype(np.int32)
        empty = np.flatnonzero(c == 0)
        if empty.size:
            idx16[empty, 0, s_off16[w] + g * (GRANULE // 16)] = 0
            c[empty] = 1
        cnts[:, i] = c

    idx_inputs = np.ascontiguousarray(np.tile(idx16, (1, 8, 1)))
    c_of = g_sorted // N_WIN
    plan = (
        tuple(gplan),
        tuple(int(x) for x in pcols),
        tuple(int(x) for x in scols),
        idx_cols,
    )
    return idx_inputs, order, src_rows, c_of, cnts, plan


_NC_CACHE = {}


def _get_nc(plan):
    if plan not in _NC_CACHE:
        _NC_CACHE[plan] = build_nc(plan)
    return _NC_CACHE[plan]


def run_sharded(indices, tables, trace=False, **spmd_kwargs):
    """Run the SPMD kernel on 8 cores; returns (full_output, BassKernelResults)."""
    from concourse import bass_utils

    tables_flat = np.asarray(tables, dtype=np.float32).reshape(F * V, D)
    idx_inputs, dst_rows, src_rows, core_of, cnts, plan = route(indices)

    in_maps = []
    for c in range(N_CORES):
        m = {"idx": idx_inputs[c], "cnt": cnts[c : c + 1]}
        shard = tables_flat[c * SHARD : (c + 1) * SHARD]
        r0 = 0
        for w in range(N_WIN):
            m[f"tab{w}"] = shard[r0 : r0 + WIN_ROWS[w]]
            r0 += WIN_ROWS[w]
        in_maps.append(m)

    nc = _get_nc(plan)
    res = bass_utils.run_bass_kernel_spmd(
        nc, in_maps, list(range(N_CORES)), trace=trace, **spmd_kwargs
    )

    out_flat = np.empty((ROWS_TOTAL, D), dtype=np.float32)
    for c in range(N_CORES):
        sel = core_of == c
        staged = np.asarray(res.results[c]["out"]).astype(np.float32)
        out_flat[dst_rows[sel]] = staged[src_rows[sel]]
    return out_flat.reshape(F, BL, D), res


def kernel(indices, tables):
    out, _ = run_sharded(indices, tables, trace=False)
    return out
